# revision 7
# baseline (speedup 1.0000x reference)
"""Causal Gaussian-kernel self-attention on 8 TRN2 NeuronCores (v2).

Reference computation (per batch b):
    qkv = x @ W_attn + b_attn ; q,k,v heads of 64 dims
    scores = exp(-(|q|^2 + |k|^2 - 2 q.k) / (2*sqrt(64))), causal-masked, NO softmax
    y = scores @ v ; out = y @ W_proj + b_proj

Sharding: core c -> batch b = c//2, head-group g = c%2 (8 heads each).
Per core the score factors:  exp(q.k/8) * exp(-|q|^2/16) * exp(-|k|^2/16)
  - exp(-|k|^2/16) is folded into the score exp via the ACT engine's
    per-partition bias operand (bias col = -|k|^2/16 in key-partition
    layout, produced by squaring k_pack and contracting 64-row groups
    with a tiny block-diagonal ones matmul -- no k-natural GEMM).
  - exp(-|q|^2/16) is folded into the y^T PSUM->SBUF copy (per-query
    scale, materialized by a ones-broadcast matmul as in v1).

v2 changes vs v1: all projection GEMMs run on bf16 inputs (fp32 PSUM
accumulate), the k-natural GEMM is gone, input DMAs are issued in
first-use order so the PE starts ~2us in, and the two heads of a pair
are emitted back-to-back for the row-tiled score matmuls and
col-tiled AV matmuls so the PE sub-arrays can overlap them.

Layouts (per core):
  xT      (1024,2048) x[b]^T bf16 (host-transposed), resident
  q_pack  4x(128,2048) head-pair q^T rows, values -2*(x@Wq+bq), fp32r
  k_pack  4x(128,2048) head-pair k^T rows, fp32r
  v~      16x(128,512) v natural, bf16
  s^T     per k-tile (128 k-rows, q-extent) exact-causal, exp'd to bf16
  y^T     4x(128,2048) head-pair packed, bf16
Row/col tile_position packs both heads of a pair into the PE array
concurrently (K=64 scores at row 0/64; M=64 AV at col 0/64).

Host side: the two head-group cores of one batch are summed (the c_proj
row-parallel all-reduce) + b_proj.
"""

import math
import os
from contextlib import ExitStack

import numpy as np
import ml_dtypes

import concourse.bass as bass
import concourse.mybir as mybir
import concourse.tile as tile
from concourse.vector_clock import ScopedClock, VectorClock
from concourse.bass_utils import run_bass_kernel_spmd

F32 = mybir.dt.float32
F32R = mybir.dt.float32r
BF16 = mybir.dt.bfloat16
AF = mybir.ActivationFunctionType
ALU = mybir.AluOpType

B, T, C, H = 4, 2048, 1024, 16
HD = C // H          # 64
HG = H // 2          # 8 heads per core
GC = HG * HD         # 512
NT = T // 128        # 16
NKC = C // 128       # 8
SCALE = -1.0 / (2.0 * math.sqrt(HD))   # -1/16

LAST_RESULTS = None
_last_in_maps = None


class _TC(tile.TileContext):
    """Tail barrier emitting one NOP per proc tick; this walrus build
    accepts only a single sync wait per instruction."""

    def _drain_and_barrier(self, tick_clock, wait_clock):
        gc = tick_clock.global_clock
        for proc in range(len(gc)):
            if gc[proc] <= 0:
                continue
            vc = VectorClock()
            vc.require_at_least(proc, gc[proc])
            nop_inst = self.nc.sync.nop(nofuse=True)
            wait_clock.add_sem_waits(nop_inst.ins, ScopedClock({None: vc}))
        self.nc.sync.drain()
        self.nc.all_engine_barrier()
        assert self.sems is not None
        popped = self.nc._tile_sem_poison_stack.pop()
        assert popped is self._sem_poison
        self.nc.clear_and_free_semaphores(list(self.sems.allocated().values()))
        self.nc.all_engine_barrier()


def _split_sync_waits(nc, keep=1):
    """Move excess per-instruction sem waits onto NOPs inserted just before,
    same engine stream (walrus here rejects >1 sync wait per instruction)."""
    for f in nc.m.functions:
        for bb in f.blocks:
            out = []
            changed = False
            for inst in bb.instructions:
                si = inst.sync_info
                waits = list(si.on_wait) if (si is not None and si.on_wait) else []
                if len(waits) > keep:
                    changed = True
                    for w in waits[:-keep]:
                        nop = mybir.InstNoOp(
                            name=f"I-wsplit-{nc.next_id()}", ins=[], outs=[]
                        )
                        nop.engine = inst.engine
                        nop.sync_info = mybir.SyncInfo(on_wait=[w], on_update=[])
                        out.append(nop)
                    ups = list(si.on_update) if si.on_update else []
                    inst.sync_info = mybir.SyncInfo(
                        on_wait=waits[-keep:], on_update=ups
                    )
                out.append(inst)
            if changed:
                bb.instructions = out


def _build_program():
    nc = bass.Bass(target_bir_lowering=False, trn_type="TRN2", debug=False)

    xT_d = nc.dram_tensor("xT", [C, T], BF16, kind="ExternalInput").ap()
    Wq_d = nc.dram_tensor("Wq", [C, GC], BF16, kind="ExternalInput").ap()
    Wk_d = nc.dram_tensor("Wk", [C, GC], BF16, kind="ExternalInput").ap()
    Wv_d = nc.dram_tensor("Wv", [C, GC], BF16, kind="ExternalInput").ap()
    bqc_d = nc.dram_tensor("bq_col", [128, 4], F32, kind="ExternalInput").ap()
    bkc_d = nc.dram_tensor("bk_col", [128, 4], F32, kind="ExternalInput").ap()
    bv_d = nc.dram_tensor("bv", [1, GC], BF16, kind="ExternalInput").ap()
    Wp_d = nc.dram_tensor("Wp", [GC, C], BF16, kind="ExternalInput").ap()
    mask_d = nc.dram_tensor("trimask", [128, 128], BF16, kind="ExternalInput").ap()
    out_d = nc.dram_tensor("out", [T, C], F32, kind="ExternalOutput").ap()

    with _TC(nc) as tc, ExitStack() as ctx:
        res = ctx.enter_context(tc.tile_pool(name="res", bufs=1))
        ps = ctx.enter_context(tc.tile_pool(name="ps", bufs=4, space="PSUM"))

        def big(nm):
            return ps.tile([128, 1024], F32, tag="big", name=nm)

        # ---- resident small tensors (scalar-engine DMA queue) ----
        mask = res.tile([128, 128], BF16, tag="mask")
        nc.scalar.dma_start(mask[:], mask_d[:])
        bqc = res.tile([128, 4], F32, tag="bqc")
        nc.scalar.dma_start(bqc[:], bqc_d[:])
        bkc = res.tile([128, 4], F32, tag="bkc")
        nc.scalar.dma_start(bkc[:], bkc_d[:])
        bv = res.tile([1, GC], BF16, tag="bv")
        nc.scalar.dma_start(bv[:], bv_d[:])
        ones_b = res.tile([1, 128], BF16, tag="ones_b")
        nc.vector.memset(ones_b[:], 1.0)
        ones_q = res.tile([128, 128], F32R, tag="ones_q")
        nc.vector.memset(ones_q[:].bitcast(F32), 0.25)
        # block-diag 64-row group reducer: col hh sums rows hh*64..+64,
        # scaled by SCALE so the contraction directly yields -|k|^2/16
        bd2 = res.tile([128, 2], F32R, tag="bd2")
        nc.vector.memset(bd2[:].bitcast(F32), 0.0)
        nc.vector.memset(bd2[0:64, 0:1].bitcast(F32), SCALE)
        nc.vector.memset(bd2[64:128, 1:2].bitcast(F32), SCALE)
        # -|k|^2/16 per key, col layout kt*8 + h  (h = 2*p + hh)
        k2b = res.tile([128, 128], F32, tag="k2b")

        # ---- resident big tensors; weights on the scalar queue, x on the
        # sync queue, both in first-use order so the two DGE rings run in
        # parallel and the first GEMM starts early ----
        xt_pool = ctx.enter_context(tc.tile_pool(name="xt", bufs=1))
        wqk_pool = ctx.enter_context(tc.tile_pool(name="wqk", bufs=1))
        xT, wq_r, wk_r = [], [], []
        for kc in range(NKC):
            wq_t = wqk_pool.tile([128, GC], BF16, tag=f"wq{kc}", name=f"wq{kc}")
            nc.scalar.dma_start(wq_t[:], Wq_d[kc * 128:(kc + 1) * 128, :])
            wq_r.append(wq_t)
            wk_t = wqk_pool.tile([128, GC], BF16, tag=f"wk{kc}", name=f"wk{kc}")
            nc.scalar.dma_start(wk_t[:], Wk_d[kc * 128:(kc + 1) * 128, :])
            wk_r.append(wk_t)
            xt_t = xt_pool.tile([128, T], BF16, tag=f"x{kc}", name=f"xT{kc}")
            nc.sync.dma_start(xt_t[:], xT_d[kc * 128:(kc + 1) * 128, :])
            xT.append(xt_t)

        qk = ctx.enter_context(tc.tile_pool(name="qk", bufs=1))
        q_pack = [qk.tile([128, T], F32R, tag=f"q{p}", name=f"q_pack{p}")
                  for p in range(4)]
        k_pack = [qk.tile([128, T], F32R, tag=f"k{p}", name=f"k_pack{p}")
                  for p in range(4)]
        vt = ctx.enter_context(tc.tile_pool(name="vt", bufs=1))
        v_sb = [vt.tile([128, GC], BF16, tag=f"v{t}", name=f"v_sb{t}")
                for t in range(NT)]
        yp = ctx.enter_context(tc.tile_pool(name="yp", bufs=1))
        y_sb = [yp.tile([128, T], BF16, tag=f"y{p}", name=f"y_sb{p}")
                for p in range(4)]
        wpp = ctx.enter_context(tc.tile_pool(name="wpp", bufs=1))
        wp = [wpp.tile([128, C], BF16, tag=f"wp{p}", name=f"wp{p}")
              for p in range(4)]

        with tc.tile_pool(name="wvp", bufs=1) as wvp, \
             tc.tile_pool(name="ksqp", bufs=2) as ksqp:

            # ==== q^T/k^T GEMMs, per T-half (one PSUM slot at a time) ====
            def emit_qk_half(p8, th):
                w_r = wq_r if p8 < 4 else wk_r
                bcol = bqc if p8 < 4 else bkc
                m0 = 128 * (p8 % 4)
                dst = q_pack[p8 % 4] if p8 < 4 else k_pack[p8 % 4]
                bigp = big(f"psqk{p8}_{th}")
                for kc in range(NKC):
                    for n in range(2):
                        nc.tensor.matmul(
                            bigp[:, n * 512:(n + 1) * 512],
                            w_r[kc][:, m0:m0 + 128],
                            xT[kc][:, (th * 2 + n) * 512:
                                     (th * 2 + n + 1) * 512],
                            start=(kc == 0), stop=(kc == NKC - 1))
                nc.vector.tensor_scalar_add(
                    dst[:, th * 1024:(th + 1) * 1024], bigp[:],
                    bcol[:, (p8 % 4):(p8 % 4) + 1])

            def emit_k2_half(p, th):
                """-|k|^2/16 for pair p's heads, key tiles of T-half th:
                square k_pack, contract 64-row groups via bd2 into a
                transient PSUM tile, strided-copy to k2b cols kt*8+2p+hh."""
                ksq = ksqp.tile([128, 1024], F32R, tag="ksq",
                                name=f"ksq{p}_{th}")
                nc.vector.tensor_mul(ksq[:], k_pack[p][:, th * 1024:
                                                       (th + 1) * 1024],
                                     k_pack[p][:, th * 1024:(th + 1) * 1024])
                k2p = big(f"k2ps{p}_{th}")
                for kt in range(8):
                    nc.tensor.matmul(
                        k2p[:, kt * 2:kt * 2 + 2],
                        ksq[:, kt * 128:(kt + 1) * 128], bd2[:],
                        start=True, stop=True)
                src = bass.AP(k2p.tensor, k2p.offset,
                              [list(k2p.ap[0]), [2, 8], [1, 2]])
                dst = bass.AP(k2b.tensor, k2b.offset + th * 64 + 2 * p,
                              [list(k2b.ap[0]), [8, 8], [1, 2]])
                nc.vector.tensor_copy(dst, src)

            wv_r = [wvp.tile([128, GC], BF16, tag=f"wv{kc}", name=f"wvr{kc}")
                    for kc in range(NKC)]
            for kc in range(NKC):
                nc.scalar.dma_start(wv_r[kc][:],
                                    Wv_d[kc * 128:(kc + 1) * 128, :])
            for p in range(4):
                nc.scalar.dma_start(wp[p][:], Wp_d[p * 128:(p + 1) * 128, :])

            def emit_v(tt):
                pv = big(f"pv{tt}")
                nc.tensor.matmul(pv[:, 0:GC], ones_b[0:1, 0:128], bv[0:1, :],
                                 start=True, stop=False)
                for kc in range(NKC):
                    nc.tensor.matmul(pv[:, 0:GC],
                                     xT[kc][:, tt * 128:tt * 128 + 128],
                                     wv_r[kc][:], start=False,
                                     stop=(kc == NKC - 1))
                nc.vector.tensor_copy(v_sb[tt][:], pv[:, 0:GC])

            emit_qk_half(0, 0)
            emit_qk_half(4, 0)
            emit_k2_half(0, 0)
            prio_mark = tc.cur_priority  # attention slots in here
            for tt in range(8):
                emit_v(tt)
            emit_qk_half(0, 1)
            emit_qk_half(4, 1)
            emit_k2_half(0, 1)
            for tt in range(8, NT):
                emit_v(tt)
            for p in (1, 2, 3):
                for th in range(2):
                    emit_qk_half(p, th)
                    emit_qk_half(p + 4, th)
                    emit_k2_half(p, th)

        # ================= attention (elevated priority) =================
        with tc.tile_pool(name="q2ep", bufs=1) as q2e_p, \
             tc.tile_pool(name="sqq", bufs=2) as sqq, \
             tc.tile_pool(name="ssb", bufs=2) as ssb, \
             tc.tile_pool(name="osb", bufs=2) as osb:
            q2eh = [q2e_p.tile([128, 1024], BF16, tag=f"e{p}", name=f"q2e{p}")
                    for p in range(4)]

            with tc.high_priority(offset=max(0, tc.cur_priority - prio_mark)):
                for half in range(2):
                    q_lo, q_hi = 1024 * half, 1024 * (half + 1)
                    for p in range(4):
                        # exp(-|q|^2/16) for this (pair, half)
                        sq_q = sqq.tile([128, 1024], F32R, tag="sqq",
                                        name=f"sqq{p}_{half}")
                        nc.vector.tensor_mul(sq_q[:], q_pack[p][:, q_lo:q_hi],
                                             q_pack[p][:, q_lo:q_hi])
                        for hh in range(2):
                            pq2 = big(f"pq2_{p}{half}{hh}")
                            for j in range(2):
                                nc.tensor.matmul(
                                    pq2[:, j * 512:(j + 1) * 512],
                                    ones_q[hh * 64:hh * 64 + 64, :],
                                    sq_q[hh * 64:hh * 64 + 64,
                                         j * 512:(j + 1) * 512],
                                    start=True, stop=True,
                                    tile_position=(hh * 64, 0))
                            nc.scalar.activation(
                                q2eh[p][hh * 64:hh * 64 + 64, :],
                                pq2[hh * 64:hh * 64 + 64, :],
                                AF.Exp, scale=SCALE)

                        y_ps = big(f"yps{p}_{half}")
                        kt_last = 8 * half + 7
                        for kt in range(kt_last + 1):
                            q0 = max(128 * kt, q_lo)
                            ext = q_hi - q0
                            s_ps = [big(f"sps{p}_{half}_{kt}_{hh}")
                                    for hh in range(2)]
                            # score matmuls: both heads back-to-back per
                            # chunk (row-tiled at 0/64 -> concurrent)
                            n0 = q0
                            while n0 < q_hi:
                                nn = min(512, q_hi - n0)
                                for hh in range(2):
                                    nc.tensor.matmul(
                                        s_ps[hh][:, n0 - q0:n0 - q0 + nn],
                                        k_pack[p][hh * 64:hh * 64 + 64,
                                                  kt * 128:kt * 128 + 128],
                                        q_pack[p][hh * 64:hh * 64 + 64,
                                                  n0:n0 + nn],
                                        start=True, stop=True,
                                        tile_position=(hh * 64, 0))
                                n0 += nn
                            s_sb = []
                            for hh in range(2):
                                h = 2 * p + hh
                                s_t = ssb.tile([128, 1024], BF16, tag=f"s{hh}",
                                               name=f"ssb{p}_{half}_{kt}_{hh}")
                                nc.scalar.activation(
                                    s_t[:, 0:ext], s_ps[hh][:, 0:ext],
                                    AF.Exp, scale=SCALE,
                                    bias=k2b[:, kt * 8 + h:kt * 8 + h + 1])
                                s_sb.append(s_t)
                            if 128 * kt >= q_lo:
                                for hh in range(2):
                                    nc.vector.tensor_mul(s_sb[hh][:, 0:128],
                                                         s_sb[hh][:, 0:128],
                                                         mask[:])
                            # AV: both heads back-to-back per col chunk
                            # (col-tiled at 0/64 -> concurrent)
                            a0 = q0
                            while a0 < q_hi:
                                a1 = min((a0 // 512 + 1) * 512, q_hi)
                                for hh in range(2):
                                    h = 2 * p + hh
                                    nc.tensor.matmul(
                                        y_ps[hh * 64:hh * 64 + 64,
                                             a0 - q_lo:a1 - q_lo],
                                        v_sb[kt][:, h * HD:h * HD + HD],
                                        s_sb[hh][:, a0 - q0:a1 - q0],
                                        start=(kt == 0), stop=(kt == kt_last),
                                        tile_position=(0, hh * 64))
                                a0 = a1
                        nc.vector.tensor_tensor(
                            y_sb[p][:, q_lo:q_hi], y_ps[:],
                            q2eh[p][:], op=ALU.mult)

                    # ---- c_proj for this T-half ----
                    for tt in range(8 * half, 8 * half + 8):
                        po = big(f"po{tt}")
                        for n2 in range(2):
                            for p4 in range(4):
                                nc.tensor.matmul(
                                    po[:, n2 * 512:(n2 + 1) * 512],
                                    y_sb[p4][:, tt * 128:tt * 128 + 128],
                                    wp[p4][:, n2 * 512:(n2 + 1) * 512],
                                    start=(p4 == 0), stop=(p4 == 3))
                        o_sb = osb.tile([128, C], F32, tag="o", name=f"osb{tt}")
                        nc.vector.tensor_copy(o_sb[:], po[:])
                        nc.sync.dma_start(out_d[tt * 128:(tt + 1) * 128, :],
                                          o_sb[:])

    _split_sync_waits(nc)
    return nc


_NC_CACHE = None


def _get_program():
    global _NC_CACHE
    if _NC_CACHE is None:
        _NC_CACHE = _build_program()
    return _NC_CACHE


def kernel(x, W_attn, b_attn, W_proj, b_proj, n_head):
    global LAST_RESULTS, _last_in_maps
    assert int(n_head) == H
    x = np.asarray(x, dtype=np.float32)
    W_attn = np.asarray(W_attn, dtype=np.float32)
    b_attn = np.asarray(b_attn, dtype=np.float32)
    W_proj = np.asarray(W_proj, dtype=np.float32)
    b_proj = np.asarray(b_proj, dtype=np.float32)

    mask = np.triu(np.ones((128, 128), np.float32)).astype(ml_dtypes.bfloat16)

    in_maps = []
    for c in range(8):
        b = c // 2
        g = c % 2
        cols = slice(g * GC, (g + 1) * GC)
        bq = -2.0 * b_attn[0 * C:1 * C][cols]
        bkv = b_attn[1 * C:2 * C][cols]
        in_maps.append({
            "xT": np.ascontiguousarray(x[b].T).astype(ml_dtypes.bfloat16),
            "Wq": np.ascontiguousarray(
                -2.0 * W_attn[:, 0 * C:1 * C][:, cols]).astype(
                    ml_dtypes.bfloat16),
            "Wk": np.ascontiguousarray(
                W_attn[:, 1 * C:2 * C][:, cols]).astype(ml_dtypes.bfloat16),
            "Wv": np.ascontiguousarray(
                W_attn[:, 2 * C:3 * C][:, cols]).astype(ml_dtypes.bfloat16),
            "bq_col": np.ascontiguousarray(bq.reshape(4, 128).T),
            "bk_col": np.ascontiguousarray(bkv.reshape(4, 128).T),
            "bv": b_attn[2 * C:3 * C][cols].reshape(1, GC).astype(
                ml_dtypes.bfloat16),
            "Wp": np.ascontiguousarray(
                W_proj[g * GC:(g + 1) * GC, :]).astype(ml_dtypes.bfloat16),
            "trimask": mask,
        })

    _last_in_maps = in_maps
    nc = _get_program()
    LAST_RESULTS = run_bass_kernel_spmd(nc, in_maps, core_ids=list(range(8)))

    out = np.empty((B, T, C), np.float32)
    for b in range(B):
        out[b] = (LAST_RESULTS.results[2 * b]["out"]
                  + LAST_RESULTS.results[2 * b + 1]["out"] + b_proj)
    return out


# revision 10
# speedup vs baseline: 1.1612x; 1.1612x over previous
"""Causal Gaussian-kernel self-attention on 8 TRN2 NeuronCores (v2).

Reference computation (per batch b):
    qkv = x @ W_attn + b_attn ; q,k,v heads of 64 dims
    scores = exp(-(|q|^2 + |k|^2 - 2 q.k) / (2*sqrt(64))), causal-masked, NO softmax
    y = scores @ v ; out = y @ W_proj + b_proj

Sharding: core c -> batch b = c//2, head-group g = c%2 (8 heads each).
Per core the score factors:  exp(q.k/8) * exp(-|q|^2/16) * exp(-|k|^2/16)
  - exp(-|k|^2/16) is folded into the score exp via the ACT engine's
    per-partition bias operand (bias col = -|k|^2/16 in key-partition
    layout, produced by squaring k_pack and contracting 64-row groups
    with a tiny block-diagonal ones matmul -- no k-natural GEMM).
  - exp(-|q|^2/16) is folded into the y^T PSUM->SBUF copy (per-query
    scale, materialized by a ones-broadcast matmul as in v1).

v2 changes vs v1: all projection GEMMs run on bf16 inputs (fp32 PSUM
accumulate), the k-natural GEMM is gone, input DMAs are issued in
first-use order so the PE starts ~2us in, and the two heads of a pair
are emitted back-to-back for the row-tiled score matmuls and
col-tiled AV matmuls so the PE sub-arrays can overlap them.

Layouts (per core):
  xT      (1024,2048) x[b]^T bf16 (host-transposed), resident
  q_pack  4x(128,2048) head-pair q^T rows, values -2*(x@Wq+bq), fp32r
  k_pack  4x(128,2048) head-pair k^T rows, fp32r
  v~      16x(128,512) v natural, bf16
  s^T     per k-tile (128 k-rows, q-extent) exact-causal, exp'd to bf16
  y^T     4x(128,2048) head-pair packed, bf16
Row/col tile_position packs both heads of a pair into the PE array
concurrently (K=64 scores at row 0/64; M=64 AV at col 0/64).

Host side: the two head-group cores of one batch are summed (the c_proj
row-parallel all-reduce) + b_proj.
"""

import math
import os
from contextlib import ExitStack

import numpy as np
import ml_dtypes

import concourse.bass as bass
import concourse.mybir as mybir
import concourse.tile as tile
from concourse.vector_clock import ScopedClock, VectorClock
from concourse.bass_utils import run_bass_kernel_spmd

F32 = mybir.dt.float32
F32R = mybir.dt.float32r
BF16 = mybir.dt.bfloat16
AF = mybir.ActivationFunctionType
ALU = mybir.AluOpType

B, T, C, H = 4, 2048, 1024, 16
HD = C // H          # 64
HG = H // 2          # 8 heads per core
GC = HG * HD         # 512
NT = T // 128        # 16
NKC = C // 128       # 8
SCALE = -1.0 / (2.0 * math.sqrt(HD))   # -1/16

LAST_RESULTS = None
_last_in_maps = None


class _TC(tile.TileContext):
    """Tail barrier emitting one NOP per proc tick; this walrus build
    accepts only a single sync wait per instruction."""

    def _drain_and_barrier(self, tick_clock, wait_clock):
        gc = tick_clock.global_clock
        for proc in range(len(gc)):
            if gc[proc] <= 0:
                continue
            vc = VectorClock()
            vc.require_at_least(proc, gc[proc])
            nop_inst = self.nc.sync.nop(nofuse=True)
            wait_clock.add_sem_waits(nop_inst.ins, ScopedClock({None: vc}))
        self.nc.sync.drain()
        self.nc.all_engine_barrier()
        assert self.sems is not None
        popped = self.nc._tile_sem_poison_stack.pop()
        assert popped is self._sem_poison
        self.nc.clear_and_free_semaphores(list(self.sems.allocated().values()))
        self.nc.all_engine_barrier()


def _split_sync_waits(nc, keep=1):
    """Move excess per-instruction sem waits onto NOPs inserted just before,
    same engine stream (walrus here rejects >1 sync wait per instruction)."""
    for f in nc.m.functions:
        for bb in f.blocks:
            out = []
            changed = False
            for inst in bb.instructions:
                si = inst.sync_info
                waits = list(si.on_wait) if (si is not None and si.on_wait) else []
                if len(waits) > keep:
                    changed = True
                    for w in waits[:-keep]:
                        nop = mybir.InstNoOp(
                            name=f"I-wsplit-{nc.next_id()}", ins=[], outs=[]
                        )
                        nop.engine = inst.engine
                        nop.sync_info = mybir.SyncInfo(on_wait=[w], on_update=[])
                        out.append(nop)
                    ups = list(si.on_update) if si.on_update else []
                    inst.sync_info = mybir.SyncInfo(
                        on_wait=waits[-keep:], on_update=ups
                    )
                out.append(inst)
            if changed:
                bb.instructions = out


def _build_program():
    nc = bass.Bass(target_bir_lowering=False, trn_type="TRN2", debug=False)

    xT_d = nc.dram_tensor("xT", [C, T], BF16, kind="ExternalInput").ap()
    Wq_d = nc.dram_tensor("Wq", [C, GC], BF16, kind="ExternalInput").ap()
    Wk_d = nc.dram_tensor("Wk", [C, GC], BF16, kind="ExternalInput").ap()
    Wv_d = nc.dram_tensor("Wv", [C, GC], BF16, kind="ExternalInput").ap()
    bqc_d = nc.dram_tensor("bq_col", [128, 4], F32, kind="ExternalInput").ap()
    bkc_d = nc.dram_tensor("bk_col", [128, 4], F32, kind="ExternalInput").ap()
    bv_d = nc.dram_tensor("bv", [1, GC], BF16, kind="ExternalInput").ap()
    Wp_d = nc.dram_tensor("Wp", [GC, C], BF16, kind="ExternalInput").ap()
    mask_d = nc.dram_tensor("trimask", [128, 128], BF16, kind="ExternalInput").ap()
    out_d = nc.dram_tensor("out", [T, C], F32, kind="ExternalOutput").ap()

    with _TC(nc) as tc, ExitStack() as ctx:
        res = ctx.enter_context(tc.tile_pool(name="res", bufs=1))
        # Three PSUM pools (8 banks total = 16KB/partition):
        #   psA 1x[128,1024] -- qkv/v/proj GEMM stream + pq2
        #   psY 1x[128,1024] -- the AV accumulator for one (pair, half)
        #   psS 2x[128,1024] -- score tiles, rotated per (kt, hh)
        # Separate pools keep the attention stream's buffer rotation
        # independent of the projection stream's, so the two phases
        # interleave on the PE instead of serializing.
        psA = ctx.enter_context(tc.tile_pool(name="psA", bufs=1, space="PSUM"))
        psY = ctx.enter_context(tc.tile_pool(name="psY", bufs=1, space="PSUM"))
        psS = ctx.enter_context(tc.tile_pool(name="psS", bufs=2, space="PSUM"))

        def bigA(nm):
            return psA.tile([128, 1024], F32, tag="A", name=nm)

        def bigY(nm):
            return psY.tile([128, 1024], F32, tag="Y", name=nm)

        def bigS(nm):
            return psS.tile([128, 1024], F32, tag="S", name=nm)

        # ---- resident small tensors (scalar-engine DMA queue) ----
        mask = res.tile([128, 128], BF16, tag="mask")
        nc.scalar.dma_start(mask[:], mask_d[:])
        bqc = res.tile([128, 4], F32, tag="bqc")
        nc.scalar.dma_start(bqc[:], bqc_d[:])
        bkc = res.tile([128, 4], F32, tag="bkc")
        nc.scalar.dma_start(bkc[:], bkc_d[:])
        bv = res.tile([1, GC], BF16, tag="bv")
        nc.scalar.dma_start(bv[:], bv_d[:])
        ones_b = res.tile([1, 128], BF16, tag="ones_b")
        nc.vector.memset(ones_b[:], 1.0)
        ones_q = res.tile([128, 128], F32R, tag="ones_q")
        nc.vector.memset(ones_q[:].bitcast(F32), 0.25)
        # block-diag 64-row group reducer: col hh sums rows hh*64..+64,
        # scaled by SCALE so the contraction directly yields -|k|^2/16
        bd2 = res.tile([128, 2], F32R, tag="bd2")
        nc.vector.memset(bd2[:].bitcast(F32), 0.0)
        nc.vector.memset(bd2[0:64, 0:1].bitcast(F32), SCALE)
        nc.vector.memset(bd2[64:128, 1:2].bitcast(F32), SCALE)
        # -|k|^2/16 per key, col layout kt*8 + h  (h = 2*p + hh)
        k2b = res.tile([128, 128], F32, tag="k2b")

        # ---- resident big tensors; weights on the scalar queue, x on the
        # sync queue, both in first-use order so the two DGE rings run in
        # parallel and the first GEMM starts early ----
        xt_pool = ctx.enter_context(tc.tile_pool(name="xt", bufs=1))
        wqk_pool = ctx.enter_context(tc.tile_pool(name="wqk", bufs=1))
        xT, wq_r, wk_r = [], [], []
        for kc in range(NKC):
            wq_t = wqk_pool.tile([128, GC], BF16, tag=f"wq{kc}", name=f"wq{kc}")
            nc.scalar.dma_start(wq_t[:], Wq_d[kc * 128:(kc + 1) * 128, :])
            wq_r.append(wq_t)
            wk_t = wqk_pool.tile([128, GC], BF16, tag=f"wk{kc}", name=f"wk{kc}")
            nc.scalar.dma_start(wk_t[:], Wk_d[kc * 128:(kc + 1) * 128, :])
            wk_r.append(wk_t)
            xt_t = xt_pool.tile([128, T], BF16, tag=f"x{kc}", name=f"xT{kc}")
            nc.sync.dma_start(xt_t[:], xT_d[kc * 128:(kc + 1) * 128, :])
            xT.append(xt_t)

        qk = ctx.enter_context(tc.tile_pool(name="qk", bufs=1))
        q_pack = [qk.tile([128, T], F32R, tag=f"q{p}", name=f"q_pack{p}")
                  for p in range(4)]
        k_pack = [qk.tile([128, T], F32R, tag=f"k{p}", name=f"k_pack{p}")
                  for p in range(4)]
        vt = ctx.enter_context(tc.tile_pool(name="vt", bufs=1))
        v_sb = [vt.tile([128, GC], BF16, tag=f"v{t}", name=f"v_sb{t}")
                for t in range(NT)]
        yp = ctx.enter_context(tc.tile_pool(name="yp", bufs=1))
        y_sb = [yp.tile([128, T], BF16, tag=f"y{p}", name=f"y_sb{p}")
                for p in range(4)]
        wpp = ctx.enter_context(tc.tile_pool(name="wpp", bufs=1))
        wp = [wpp.tile([128, C], BF16, tag=f"wp{p}", name=f"wp{p}")
              for p in range(4)]

        wvp = ctx.enter_context(tc.tile_pool(name="wvp", bufs=1))
        ksqp = ctx.enter_context(tc.tile_pool(name="ksqp", bufs=2))
        q2e_p = ctx.enter_context(tc.tile_pool(name="q2ep", bufs=1))
        sqq = ctx.enter_context(tc.tile_pool(name="sqq", bufs=2))
        ssb = ctx.enter_context(tc.tile_pool(name="ssb", bufs=2))
        osb = ctx.enter_context(tc.tile_pool(name="osb", bufs=2))
        q2eh = [q2e_p.tile([128, 1024], BF16, tag=f"e{p}", name=f"q2e{p}")
                for p in range(4)]

        wv_r = [wvp.tile([128, GC], BF16, tag=f"wv{kc}", name=f"wvr{kc}")
                for kc in range(NKC)]
        for kc in range(NKC):
            nc.scalar.dma_start(wv_r[kc][:], Wv_d[kc * 128:(kc + 1) * 128, :])
        for p in range(4):
            nc.scalar.dma_start(wp[p][:], Wp_d[p * 128:(p + 1) * 128, :])

        # ==== q^T/k^T GEMMs, per T-half (one psA slot at a time) ====
        def emit_qk_half(p8, th):
            w_r = wq_r if p8 < 4 else wk_r
            bcol = bqc if p8 < 4 else bkc
            m0 = 128 * (p8 % 4)
            dst = q_pack[p8 % 4] if p8 < 4 else k_pack[p8 % 4]
            bigp = bigA(f"psqk{p8}_{th}")
            for kc in range(NKC):
                for n in range(2):
                    nc.tensor.matmul(
                        bigp[:, n * 512:(n + 1) * 512],
                        w_r[kc][:, m0:m0 + 128],
                        xT[kc][:, (th * 2 + n) * 512:(th * 2 + n + 1) * 512],
                        start=(kc == 0), stop=(kc == NKC - 1))
            nc.vector.tensor_scalar_add(
                dst[:, th * 1024:(th + 1) * 1024], bigp[:],
                bcol[:, (p8 % 4):(p8 % 4) + 1])

        def emit_k2_half(p, th):
            """-|k|^2/16 for pair p's heads, key tiles of T-half th:
            square k_pack, contract 64-row groups via bd2 into a
            transient PSUM tile, strided-copy to k2b cols kt*8+2p+hh."""
            ksq = ksqp.tile([128, 1024], F32R, tag="ksq",
                            name=f"ksq{p}_{th}")
            nc.vector.tensor_mul(ksq[:], k_pack[p][:, th * 1024:
                                                   (th + 1) * 1024],
                                 k_pack[p][:, th * 1024:(th + 1) * 1024])
            k2p = bigS(f"k2ps{p}_{th}")
            for kt in range(8):
                nc.tensor.matmul(
                    k2p[:, kt * 2:kt * 2 + 2],
                    ksq[:, kt * 128:(kt + 1) * 128], bd2[:],
                    start=True, stop=True)
            src = bass.AP(k2p.tensor, k2p.offset,
                          [list(k2p.ap[0]), [2, 8], [1, 2]])
            dst = bass.AP(k2b.tensor, k2b.offset + th * 64 + 2 * p,
                          [list(k2b.ap[0]), [8, 8], [1, 2]])
            nc.vector.tensor_copy(dst, src)

        def emit_v(tt):
            pv = bigA(f"pv{tt}")
            nc.tensor.matmul(pv[:, 0:GC], ones_b[0:1, 0:128], bv[0:1, :],
                             start=True, stop=False)
            for kc in range(NKC):
                nc.tensor.matmul(pv[:, 0:GC],
                                 xT[kc][:, tt * 128:tt * 128 + 128],
                                 wv_r[kc][:], start=False,
                                 stop=(kc == NKC - 1))
            nc.vector.tensor_copy(v_sb[tt][:], pv[:, 0:GC])

        def emit_att(p, half):
            q_lo, q_hi = 1024 * half, 1024 * (half + 1)
            # exp(-|q|^2/16): one pq2 tile, both heads via disjoint
            # row/col quadrants (concurrent), one full-width exp
            sq_q = sqq.tile([128, 1024], F32R, tag="sqq",
                            name=f"sqq{p}_{half}")
            nc.vector.tensor_mul(sq_q[:], q_pack[p][:, q_lo:q_hi],
                                 q_pack[p][:, q_lo:q_hi])
            for hh in range(2):
                pq2 = bigA(f"pq2_{p}{half}{hh}")
                for j in range(2):
                    nc.tensor.matmul(
                        pq2[:, j * 512:(j + 1) * 512],
                        ones_q[hh * 64:hh * 64 + 64, :],
                        sq_q[hh * 64:hh * 64 + 64, j * 512:(j + 1) * 512],
                        start=True, stop=True,
                        tile_position=(hh * 64, 0))
                nc.scalar.activation(
                    q2eh[p][hh * 64:hh * 64 + 64, :],
                    pq2[hh * 64:hh * 64 + 64, :], AF.Exp, scale=SCALE)

            y_ps = bigY(f"yps{p}_{half}")
            kt_last = 8 * half + 7
            for kt in range(kt_last + 1):
                q0 = max(128 * kt, q_lo)
                ext = q_hi - q0
                s_ps = [bigS(f"sps{p}_{half}_{kt}_{hh}") for hh in range(2)]
                # score matmuls: both heads back-to-back per chunk
                # (row-tiled at 0/64 -> concurrent)
                n0 = q0
                while n0 < q_hi:
                    nn = min(512, q_hi - n0)
                    for hh in range(2):
                        nc.tensor.matmul(
                            s_ps[hh][:, n0 - q0:n0 - q0 + nn],
                            k_pack[p][hh * 64:hh * 64 + 64,
                                      kt * 128:kt * 128 + 128],
                            q_pack[p][hh * 64:hh * 64 + 64, n0:n0 + nn],
                            start=True, stop=True,
                            tile_position=(hh * 64, 0))
                    n0 += nn
                s_sb = []
                for hh in range(2):
                    h = 2 * p + hh
                    s_t = ssb.tile([128, 1024], BF16, tag=f"s{hh}",
                                   name=f"ssb{p}_{half}_{kt}_{hh}")
                    nc.scalar.activation(
                        s_t[:, 0:ext], s_ps[hh][:, 0:ext],
                        AF.Exp, scale=SCALE,
                        bias=k2b[:, kt * 8 + h:kt * 8 + h + 1])
                    s_sb.append(s_t)
                if 128 * kt >= q_lo:
                    for hh in range(2):
                        nc.gpsimd.tensor_mul(s_sb[hh][:, 0:128],
                                             s_sb[hh][:, 0:128],
                                             mask[:])
                # AV: both heads back-to-back per col chunk
                # (col-tiled at 0/64 -> concurrent)
                a0 = q0
                while a0 < q_hi:
                    a1 = min((a0 // 512 + 1) * 512, q_hi)
                    for hh in range(2):
                        h = 2 * p + hh
                        nc.tensor.matmul(
                            y_ps[hh * 64:hh * 64 + 64,
                                 a0 - q_lo:a1 - q_lo],
                            v_sb[kt][:, h * HD:h * HD + HD],
                            s_sb[hh][:, a0 - q0:a1 - q0],
                            start=(kt == 0), stop=(kt == kt_last),
                            tile_position=(0, hh * 64))
                    a0 = a1
            nc.vector.tensor_tensor(
                y_sb[p][:, q_lo:q_hi], y_ps[:], q2eh[p][:], op=ALU.mult)

        def emit_proj(tt):
            po = bigA(f"po{tt}")
            for n2 in range(2):
                for p4 in range(4):
                    nc.tensor.matmul(
                        po[:, n2 * 512:(n2 + 1) * 512],
                        y_sb[p4][:, tt * 128:tt * 128 + 128],
                        wp[p4][:, n2 * 512:(n2 + 1) * 512],
                        start=(p4 == 0), stop=(p4 == 3))
            o_sb = osb.tile([128, C], F32, tag="o", name=f"osb{tt}")
            nc.vector.tensor_copy(o_sb[:], po[:])
            nc.sync.dma_start(out_d[tt * 128:(tt + 1) * 128, :], o_sb[:])

        # ---- interleaved emission: attention follows its producers so
        # the PSUM rotations of the two streams overlap on the PE ----
        emit_qk_half(0, 0)
        emit_qk_half(4, 0)
        emit_k2_half(0, 0)
        for tt in range(8):
            emit_v(tt)
        emit_att(0, 0)
        for p in (1, 2, 3):
            emit_qk_half(p, 0)
            emit_qk_half(p + 4, 0)
            emit_k2_half(p, 0)
            emit_att(p, 0)
        emit_qk_half(0, 1)
        emit_qk_half(4, 1)
        emit_k2_half(0, 1)
        for tt in range(8, NT):
            emit_v(tt)
        for tt in range(8):
            emit_proj(tt)
        emit_att(0, 1)
        for p in (1, 2, 3):
            emit_qk_half(p, 1)
            emit_qk_half(p + 4, 1)
            emit_k2_half(p, 1)
            emit_att(p, 1)
        for tt in range(8, NT):
            emit_proj(tt)

    _split_sync_waits(nc)
    return nc


_NC_CACHE = None


def _get_program():
    global _NC_CACHE
    if _NC_CACHE is None:
        _NC_CACHE = _build_program()
    return _NC_CACHE


def kernel(x, W_attn, b_attn, W_proj, b_proj, n_head):
    global LAST_RESULTS, _last_in_maps
    assert int(n_head) == H
    x = np.asarray(x, dtype=np.float32)
    W_attn = np.asarray(W_attn, dtype=np.float32)
    b_attn = np.asarray(b_attn, dtype=np.float32)
    W_proj = np.asarray(W_proj, dtype=np.float32)
    b_proj = np.asarray(b_proj, dtype=np.float32)

    mask = np.triu(np.ones((128, 128), np.float32)).astype(ml_dtypes.bfloat16)

    in_maps = []
    for c in range(8):
        b = c // 2
        g = c % 2
        cols = slice(g * GC, (g + 1) * GC)
        bq = -2.0 * b_attn[0 * C:1 * C][cols]
        bkv = b_attn[1 * C:2 * C][cols]
        in_maps.append({
            "xT": np.ascontiguousarray(x[b].T).astype(ml_dtypes.bfloat16),
            "Wq": np.ascontiguousarray(
                -2.0 * W_attn[:, 0 * C:1 * C][:, cols]).astype(
                    ml_dtypes.bfloat16),
            "Wk": np.ascontiguousarray(
                W_attn[:, 1 * C:2 * C][:, cols]).astype(ml_dtypes.bfloat16),
            "Wv": np.ascontiguousarray(
                W_attn[:, 2 * C:3 * C][:, cols]).astype(ml_dtypes.bfloat16),
            "bq_col": np.ascontiguousarray(bq.reshape(4, 128).T),
            "bk_col": np.ascontiguousarray(bkv.reshape(4, 128).T),
            "bv": b_attn[2 * C:3 * C][cols].reshape(1, GC).astype(
                ml_dtypes.bfloat16),
            "Wp": np.ascontiguousarray(
                W_proj[g * GC:(g + 1) * GC, :]).astype(ml_dtypes.bfloat16),
            "trimask": mask,
        })

    _last_in_maps = in_maps
    nc = _get_program()
    LAST_RESULTS = run_bass_kernel_spmd(nc, in_maps, core_ids=list(range(8)))

    out = np.empty((B, T, C), np.float32)
    for b in range(B):
        out[b] = (LAST_RESULTS.results[2 * b]["out"]
                  + LAST_RESULTS.results[2 * b + 1]["out"] + b_proj)
    return out
